# revision 3
# baseline (speedup 1.0000x reference)
"""GraphSAGE supervised 2-layer forward on 8 trn2 NeuronCores.

Data-parallel over the batch of root ids: each core handles B/8 = 512 roots.
feats table (bf16) + weights replicated per core; all gathers via SWDGE
indirect DMA with multi-offset instructions (one DMA inst covers a 5-chunk
block = 6400 rows for l2 / 640 rows for l1) to amortize the ~1us per-inst
SWDGE descriptor-generation cost that bound the v1 kernel.

Per-core dataflow (BC=512 roots, L1=12800 l1-nodes, L2=128000 l2-nodes):
  loop over 20 blocks of 5 chunks (chunk = 128 l1-nodes):
    one gather h2 [128, 50*256] bf16 (partition p = l1-node, neighbor rows
      side by side in free dim) + one gather h1 [128, 5*256] bf16
    per chunk: agg2 = tree-sum of the 10 neighbor rows (DVE, 4 adds)
      h1T, agg2T via PE transpose (bf16) -> ACT copy to SBUF
      a1 = relu([h1@wx1, agg2@wn1/10]) (PE bf16 -> PSUM f32, ACT relu -> bf16)
      psum_acc += sumB_j.T @ h1 ; psum_acc2 += sumB_j.T @ a1  (segment sum
        over 25; the 1/25 mean is folded into wn1/25 and wn2/25 weights)
  per supercycle of 5 blocks (=128 roots): a0, b0, row-normalize (f32), @w_fc
"""

import os
import numpy as np

P = 128
S1, S2 = 25, 10
D = 256          # D_IN and 2*D_HID
H = 128          # D_HID
NCLS = 40
NCORES = 8
NNODES = 100000
BLK = 5          # chunks per gather block

_programs = {}
NQUEUES = 2


def _build_program(BC):
    """Build + compile the SPMD bass program for BC roots per core."""
    from contextlib import ExitStack

    import concourse.bacc as bacc
    import concourse.tile as tile
    from concourse import bass, mybir

    L1 = BC * S1                      # l1 nodes per core
    NCH = L1 // P                     # chunks of 128 l1-nodes
    NSC = BC // P                     # supercycles (128 roots each)
    CPS = NCH // NSC                  # chunks per supercycle (25)
    NBLK = NCH // BLK                 # gather blocks
    BPS = CPS // BLK                  # blocks per supercycle
    assert L1 % P == 0 and BC % P == 0 and NCH % NSC == 0
    assert NCH % BLK == 0 and CPS % BLK == 0

    f32 = mybir.dt.float32
    bf16 = mybir.dt.bfloat16
    i32 = mybir.dt.int32
    AF = mybir.ActivationFunctionType

    nc = bacc.Bacc(
        "TRN2", target_bir_lowering=False, debug=False, num_devices=NCORES,
        num_swdge_queues=NQUEUES,
    )

    def gather(out_ap, off_ap, feats_ap, qn=0):
        """Indirect row-gather on SWDGE queue qn."""
        inst = nc.gpsimd.indirect_dma_start(
            out=out_ap, out_offset=None, in_=feats_ap,
            in_offset=bass.IndirectOffsetOnAxis(ap=off_ap, axis=0))
        if qn:
            inst.ins.queue = f"qPoolDynamic{qn}"
        return inst

    feats = nc.dram_tensor("feats", [NNODES, D], bf16, kind="ExternalInput").ap()
    idx0 = nc.dram_tensor("idx0", [P, NSC], i32, kind="ExternalInput").ap()
    idx1 = nc.dram_tensor("idx1", [P, NCH], i32, kind="ExternalInput").ap()
    idx2 = nc.dram_tensor("idx2", [P, NCH * S2], i32, kind="ExternalInput").ap()
    # weight stack: [wx1_0,wx1_1, wn1d25_0,_1, wn1d10_0,_1, wx2_0,_1,
    #               wn2d25_0,_1] each [128,128] bf16 (K-major halves,
    # pre-transposed on host so slot k holds rows [128k:128k+128])
    w5 = nc.dram_tensor("w5", [P, 10 * H], bf16, kind="ExternalInput").ap()
    wfc = nc.dram_tensor("wfc", [P, 2 * NCLS], f32, kind="ExternalInput").ap()
    bfc = nc.dram_tensor("bfc", [P, NCLS], f32, kind="ExternalInput").ap()
    sumB = nc.dram_tensor("sumB", [P, CPS * P], bf16, kind="ExternalInput").ap()
    identb = nc.dram_tensor("identb", [P, P], bf16, kind="ExternalInput").ap()
    identf = nc.dram_tensor("identf", [P, P], f32, kind="ExternalInput").ap()
    out = nc.dram_tensor("out", [BC, NCLS], f32, kind="ExternalOutput").ap()

    with tile.TileContext(nc) as tc, ExitStack() as ctx:
        consts = ctx.enter_context(tc.tile_pool(name="consts", bufs=1))
        p_h2 = ctx.enter_context(tc.tile_pool(name="h2", bufs=3))
        p_h1 = ctx.enter_context(tc.tile_pool(name="h1", bufs=3))
        p_a1 = ctx.enter_context(tc.tile_pool(name="a1", bufs=3))
        p_agg = ctx.enter_context(tc.tile_pool(name="agg", bufs=3))
        p_t = ctx.enter_context(tc.tile_pool(name="tsb", bufs=3))
        p_misc = ctx.enter_context(tc.tile_pool(name="misc", bufs=2))
        ps_tr1 = ctx.enter_context(tc.tile_pool(name="ps_tr1", bufs=2, space="PSUM"))
        ps_tr2 = ctx.enter_context(tc.tile_pool(name="ps_tr2", bufs=2, space="PSUM"))
        ps_mm = ctx.enter_context(tc.tile_pool(name="ps_mm", bufs=2, space="PSUM"))
        ps_acc = ctx.enter_context(tc.tile_pool(name="ps_acc", bufs=1, space="PSUM"))
        ps_out = ctx.enter_context(tc.tile_pool(name="ps_out", bufs=1, space="PSUM"))

        # ---- preload constants ----
        sb_idx0 = consts.tile([P, NSC], i32)
        sb_idx1 = consts.tile([P, NCH], i32)
        sb_idx2 = consts.tile([P, NCH * S2], i32)
        sb_w5 = consts.tile([P, 10 * H], bf16)
        sb_wfc = consts.tile([P, 2 * NCLS], f32)
        sb_bfc = consts.tile([P, NCLS], f32)
        sb_sumB = consts.tile([P, CPS * P], bf16)
        sb_idb = consts.tile([P, P], bf16)
        sb_idf = consts.tile([P, P], f32)
        nc.sync.dma_start(sb_idx0[:], idx0[:])
        nc.sync.dma_start(sb_idx1[:], idx1[:])
        nc.sync.dma_start(sb_idx2[:], idx2[:])
        nc.sync.dma_start(sb_w5[:], w5[:])
        nc.sync.dma_start(sb_wfc[:], wfc[:])
        nc.sync.dma_start(sb_bfc[:], bfc[:])
        nc.sync.dma_start(sb_sumB[:], sumB[:])
        nc.sync.dma_start(sb_idb[:], identb[:])
        nc.sync.dma_start(sb_idf[:], identf[:])

        def wslot(k):
            return sb_w5[:, k * H:(k + 1) * H]

        # slots: wx1=0,1  wn1/25=2,3  wn1/10=4,5  wx2=6,7  wn2/25=8,9

        def transpose256(src, tag_ps, tag_sb, dt, ident):
            """[128, 256] row-major -> [128, 256] where [:, 128k:...] holds
            the transpose of src's k-th feature half (i.e. feature-major)."""
            ps = tag_ps.tile([P, 2 * P], dt)
            nc.tensor.transpose(ps[:, 0:P], src[:, 0:P], ident)
            nc.tensor.transpose(ps[:, P:2 * P], src[:, P:2 * P], ident)
            sb = tag_sb.tile([P, 2 * P], dt)
            nc.scalar.copy(sb[:], ps[:])
            return sb

        def trb(src, tag_ps):
            return transpose256(src, tag_ps, p_t, bf16, sb_idb[:])

        def mm_pair(out_ps, xT, w0, w1):
            nc.tensor.matmul(out=out_ps, lhsT=xT[:, 0:P], rhs=w0,
                             start=True, stop=False)
            nc.tensor.matmul(out=out_ps, lhsT=xT[:, P:2 * P], rhs=w1,
                             start=False, stop=True)

        for s in range(NSC):
            acc = ps_acc.tile([P, 4 * P], f32, tag="acc")  # [sum_h1 | sum_a1]
            for bb in range(BPS):
                b = s * BPS + bb
                # ---- block gathers: 5 chunks at once ----
                h2t = p_h2.tile([P, BLK * S2 * D], bf16, tag="h2")
                gather(h2t[:], sb_idx2[:, b * BLK * S2:(b + 1) * BLK * S2],
                       feats[:], qn=b % NQUEUES)
                h1t = p_h1.tile([P, BLK * D], bf16, tag="h1")
                gather(h1t[:], sb_idx1[:, b * BLK:(b + 1) * BLK], feats[:],
                       qn=(b + 1) % NQUEUES)
                for cb in range(BLK):
                    j = bb * BLK + cb          # chunk index within supercycle
                    base = cb * S2 * D
                    h1c = h1t[:, cb * D:(cb + 1) * D]
                    # ---- agg2 = tree-sum of the 10 neighbor rows ----
                    u = p_agg.tile([P, 5 * D], bf16, tag="u")
                    nc.vector.tensor_add(u[:], h2t[:, base:base + 5 * D],
                                         h2t[:, base + 5 * D:base + 10 * D])
                    v = p_agg.tile([P, 2 * D], bf16, tag="v")
                    nc.vector.tensor_add(v[:], u[:, 0:2 * D], u[:, 2 * D:4 * D])
                    agg2 = p_agg.tile([P, D], bf16, tag="agg2")
                    nc.vector.tensor_add(agg2[:], v[:, 0:D], v[:, D:2 * D])
                    nc.vector.tensor_add(agg2[:], agg2[:], u[:, 4 * D:5 * D])
                    # ---- transposes ----
                    h1T = trb(h1c, ps_tr1)
                    agg2T = trb(agg2[:], ps_tr2)
                    # ---- a1 = relu([h1@wx1, agg2@wn1/10]) ----
                    a1ps = ps_mm.tile([P, D], f32, tag="a1ps")
                    mm_pair(a1ps[:, 0:H], h1T, wslot(0), wslot(1))
                    mm_pair(a1ps[:, H:D], agg2T, wslot(4), wslot(5))
                    a1t = p_a1.tile([P, D], bf16, tag="a1")
                    nc.scalar.activation(a1t[:], a1ps[:], AF.Relu)
                    # ---- segment-sum accumulators (over 25 chunks) ----
                    nc.tensor.matmul(
                        out=acc[:, 0:2 * P], lhsT=sb_sumB[:, j * P:(j + 1) * P],
                        rhs=h1c, start=(j == 0), stop=(j == CPS - 1),
                        skip_group_check=True,
                    )
                    nc.tensor.matmul(
                        out=acc[:, 2 * P:4 * P],
                        lhsT=sb_sumB[:, j * P:(j + 1) * P],
                        rhs=a1t[:], start=(j == 0), stop=(j == CPS - 1),
                        skip_group_check=True,
                    )

            # ---- supercycle tail: 128 roots ----
            h0t = p_misc.tile([P, D], bf16, tag="h0")
            gather(h0t[:], sb_idx0[:, s:s + 1], feats[:], qn=s % NQUEUES)
            aggs = p_misc.tile([P, 2 * D], bf16, tag="aggs")
            nc.vector.tensor_copy(aggs[:], acc[:])

            h0T = trb(h0t[:], ps_tr1)
            agg1T = trb(aggs[:, 0:D], ps_tr2)
            a0ps = ps_mm.tile([P, D], f32, tag="a1ps")
            mm_pair(a0ps[:, 0:H], h0T, wslot(0), wslot(1))
            mm_pair(a0ps[:, H:D], agg1T, wslot(2), wslot(3))
            a0t = p_misc.tile([P, D], bf16, tag="a0")
            nc.scalar.activation(a0t[:], a0ps[:], AF.Relu)

            a0T = trb(a0t[:], ps_tr1)
            aggA1T = trb(aggs[:, D:2 * D], ps_tr2)
            b0ps = ps_mm.tile([P, D], f32, tag="a1ps")
            mm_pair(b0ps[:, 0:H], a0T, wslot(6), wslot(7))
            mm_pair(b0ps[:, H:D], aggA1T, wslot(8), wslot(9))
            b0t = p_misc.tile([P, D], f32, tag="b0")
            nc.scalar.activation(b0t[:], b0ps[:], AF.Relu)

            # ---- row-normalize (F.normalize with eps=1e-12), f32 ----
            sq = p_misc.tile([P, D], f32, tag="sq")
            ss = p_misc.tile([P, 4], f32, tag="ss")
            nc.scalar.activation(sq[:], b0t[:], AF.Square,
                                 accum_out=ss[:, 0:1])
            nc.vector.tensor_scalar_max(ss[:, 1:2], ss[:, 0:1], 1e-24)
            nc.scalar.sqrt(ss[:, 2:3], ss[:, 1:2])
            nc.vector.reciprocal(ss[:, 3:4], ss[:, 2:3])
            b0n = p_misc.tile([P, D], f32, tag="b0n")
            nc.vector.tensor_scalar_mul(b0n[:], b0t[:], ss[:, 3:4])

            # ---- classifier (f32) ----
            b0nT = transpose256(b0n[:], ps_tr1, p_t, f32, sb_idf[:])
            ops = ps_out.tile([P, NCLS], f32, tag="ops")
            nc.tensor.matmul(out=ops[:], lhsT=b0nT[:, 0:P],
                             rhs=sb_wfc[:, 0:NCLS], start=True, stop=False)
            nc.tensor.matmul(out=ops[:], lhsT=b0nT[:, P:2 * P],
                             rhs=sb_wfc[:, NCLS:2 * NCLS],
                             start=False, stop=True)
            osb = p_misc.tile([P, NCLS], f32, tag="osb")
            nc.vector.tensor_add(osb[:], ops[:], sb_bfc[:])
            nc.sync.dma_start(out[s * P:(s + 1) * P, :], osb[:])

    nc.compile()
    return nc


def _get_program(BC):
    if BC not in _programs:
        _programs[BC] = _build_program(BC)
    return _programs[BC]


def _to_bf16(a):
    import jax.numpy as jnp
    return np.asarray(jnp.asarray(np.asarray(a, np.float32), jnp.bfloat16))


def _host_prep_shared(wx1, wn1, wx2, wn2, w_fc, b_fc, CPS):
    """Weight/constant tensors shared by all cores, in SBUF-ready layouts."""
    def kmaj(w):  # [256,128] -> [128, 2, 128] halves of the K dim
        return np.ascontiguousarray(
            w.reshape(2, P, -1).transpose(1, 0, 2), np.float32)

    w5 = np.concatenate(
        [kmaj(wx1), kmaj(wn1 / S1), kmaj(wn1 / S2), kmaj(wx2),
         kmaj(wn2 / S1)], axis=1
    ).reshape(P, 10 * H)
    w5 = _to_bf16(w5)
    wfc = kmaj(w_fc).reshape(P, 2 * NCLS)
    bfc = np.ascontiguousarray(np.tile(b_fc.reshape(1, NCLS), (P, 1)), np.float32)
    # sumB matrices: row p of chunk j belongs to local root (128j+p)//25;
    # entry 1.0 makes the accumulated matmul a sum over 25 (mean folded
    # into the /25-scaled weights).
    sumB = np.zeros((P, CPS, P), np.float32)
    for j in range(CPS):
        for p in range(P):
            sumB[p, j, (P * j + p) // S1] = 1.0
    sumB = _to_bf16(np.ascontiguousarray(sumB.reshape(P, CPS * P)))
    identb = _to_bf16(np.eye(P, dtype=np.float32))
    identf = np.eye(P, dtype=np.float32)
    return w5, wfc, bfc, sumB, identb, identf


def make_in_maps(ids, ids_l1, ids_l2, feats, wx1, wn1, wx2, wn2, w_fc, b_fc):
    ids = np.ascontiguousarray(np.asarray(ids), np.int32).ravel()
    ids_l1 = np.ascontiguousarray(np.asarray(ids_l1), np.int32).ravel()
    ids_l2 = np.ascontiguousarray(np.asarray(ids_l2), np.int32).ravel()
    feats = _to_bf16(feats)
    wx1, wn1, wx2, wn2, w_fc, b_fc = (
        np.asarray(a, np.float32) for a in (wx1, wn1, wx2, wn2, w_fc, b_fc))

    B = ids.shape[0]
    BC = B // NCORES
    L1, L2 = BC * S1, BC * S1 * S2
    NCH = L1 // P
    NSC = BC // P
    CPS = NCH // NSC

    w5, wfc, bfc, sumB, identb, identf = _host_prep_shared(
        wx1, wn1, wx2, wn2, w_fc, b_fc, CPS)

    in_maps = []
    for c in range(NCORES):
        i0 = ids[c * BC:(c + 1) * BC]
        i1 = ids_l1[c * L1:(c + 1) * L1]
        i2 = ids_l2[c * L2:(c + 1) * L2]
        in_maps.append({
            "feats": feats,
            "idx0": np.ascontiguousarray(i0.reshape(NSC, P).T),
            "idx1": np.ascontiguousarray(i1.reshape(NCH, P).T),
            "idx2": np.ascontiguousarray(
                i2.reshape(NCH, P, S2).transpose(1, 0, 2).reshape(P, NCH * S2)),
            "w5": w5, "wfc": wfc, "bfc": bfc, "sumB": sumB,
            "identb": identb, "identf": identf,
        })
    return in_maps, BC


def kernel(ids, ids_l1, ids_l2, feats, wx1, wn1, wx2, wn2, w_fc, b_fc):
    from concourse.bass_utils import run_bass_kernel_spmd

    in_maps, BC = make_in_maps(
        ids, ids_l1, ids_l2, feats, wx1, wn1, wx2, wn2, w_fc, b_fc)
    nc = _get_program(BC)
    res = run_bass_kernel_spmd(
        nc, in_maps, list(range(NCORES)),
        trace=bool(os.environ.get("KERNEL_TRACE")),
    )
    global LAST_RESULTS
    LAST_RESULTS = res
    return np.concatenate(
        [res.results[c]["out"] for c in range(NCORES)], axis=0
    ).astype(np.float32)


LAST_RESULTS = None


# revision 4
# speedup vs baseline: 1.2993x; 1.2993x over previous
"""GraphSAGE supervised 2-layer forward on 8 trn2 NeuronCores.

Data-parallel over the batch of root ids: each core handles B/8 = 512 roots.
feats table + weights replicated per core; all gathers via SWDGE indirect DMA.

Per-core dataflow (BC=512 roots, L1=12800 l1-nodes, L2=128000 l2-nodes):
  loop over 100 chunks of 128 l1-nodes:
    gather h2 [128, 10*256] (group-major: partition p = l1-node, 10 neighbor
      rows side by side in free dim) + h1 [128, 256]
    agg2 = sum_j h2[:, j]          (DVE, 9 adds; 1/10 folded into wn1s)
    h1T, agg2T via PE transpose    (for use as matmul lhsT)
    a1 = relu([h1@wx1, agg2@wn1s]) (PE -> PSUM, ACT relu -> SBUF)
    psum_acc += B_j.T @ [h1 | a1]  (PE accumulate: segment-mean over 25 ->
                                    agg1 = mean25(h1), agg_a1 = mean25(a1))
  per supercycle of 25 chunks (=128 roots): a0, b0, row-normalize, @w_fc
"""

import os
import numpy as np

P = 128
S1, S2 = 25, 10
D = 256          # D_IN and 2*D_HID
H = 128          # D_HID
NCLS = 40
NCORES = 8
NNODES = 100000

_programs = {}
NQUEUES = 1


def _build_program(BC):
    """Build + compile the SPMD bass program for BC roots per core."""
    from contextlib import ExitStack

    import concourse.bacc as bacc
    import concourse.tile as tile
    from concourse import bass, mybir

    L1 = BC * S1                      # l1 nodes per core
    NCH = L1 // P                     # chunks of 128 l1-nodes
    NSC = BC // P                     # supercycles (128 roots each)
    CPS = NCH // NSC                  # chunks per supercycle (25)
    assert L1 % P == 0 and BC % P == 0 and NCH % NSC == 0

    f32 = mybir.dt.float32
    i32 = mybir.dt.int32
    AF = mybir.ActivationFunctionType

    nc = bacc.Bacc(
        "TRN2", target_bir_lowering=False, debug=False, num_devices=NCORES,
        num_swdge_queues=NQUEUES,
    )
    qrr = [0]

    def gather(out_ap, off_ap, feats_ap):
        """Indirect row-gather, round-robined across the SWDGE queues."""
        inst = nc.gpsimd.indirect_dma_start(
            out=out_ap, out_offset=None, in_=feats_ap,
            in_offset=bass.IndirectOffsetOnAxis(ap=off_ap, axis=0))
        qn = qrr[0] % NQUEUES
        qrr[0] += 1
        if qn:
            inst.ins.queue = f"qPoolDynamic{qn}"
        return inst

    feats = nc.dram_tensor("feats", [NNODES, D], f32, kind="ExternalInput").ap()
    idx0 = nc.dram_tensor("idx0", [P, NSC], i32, kind="ExternalInput").ap()
    idx1 = nc.dram_tensor("idx1", [P, NCH], i32, kind="ExternalInput").ap()
    idx2 = nc.dram_tensor("idx2", [P, NCH * S2], i32, kind="ExternalInput").ap()
    # weight stack: [wx1_0,wx1_1, wn1_0,wn1_1, wn1s_0,wn1s_1, wx2_0,wx2_1,
    #               wn2_0,wn2_1] each [128,128] (K-major halves, pre-transposed
    # on host so slot k holds rows [128k:128k+128] of the [256,128] matrix)
    w5 = nc.dram_tensor("w5", [P, 10 * H], f32, kind="ExternalInput").ap()
    wfc = nc.dram_tensor("wfc", [P, 2 * NCLS], f32, kind="ExternalInput").ap()
    bfc = nc.dram_tensor("bfc", [P, NCLS], f32, kind="ExternalInput").ap()
    avgB = nc.dram_tensor("avgB", [P, CPS * P], f32, kind="ExternalInput").ap()
    ident = nc.dram_tensor("ident", [P, P], f32, kind="ExternalInput").ap()
    out = nc.dram_tensor("out", [BC, NCLS], f32, kind="ExternalOutput").ap()

    with tile.TileContext(nc) as tc, ExitStack() as ctx:
        consts = ctx.enter_context(tc.tile_pool(name="consts", bufs=1))
        p_h2 = ctx.enter_context(tc.tile_pool(name="h2", bufs=4))
        p_ha = ctx.enter_context(tc.tile_pool(name="ha", bufs=3))
        p_agg2 = ctx.enter_context(tc.tile_pool(name="agg2", bufs=3))
        p_t = ctx.enter_context(tc.tile_pool(name="tsb", bufs=3))
        p_misc = ctx.enter_context(tc.tile_pool(name="misc", bufs=2))
        ps_tr1 = ctx.enter_context(tc.tile_pool(name="ps_tr1", bufs=2, space="PSUM"))
        ps_tr2 = ctx.enter_context(tc.tile_pool(name="ps_tr2", bufs=2, space="PSUM"))
        ps_mm = ctx.enter_context(tc.tile_pool(name="ps_mm", bufs=2, space="PSUM"))
        ps_acc = ctx.enter_context(tc.tile_pool(name="ps_acc", bufs=1, space="PSUM"))
        ps_out = ctx.enter_context(tc.tile_pool(name="ps_out", bufs=1, space="PSUM"))

        # ---- preload constants ----
        sb_idx0 = consts.tile([P, NSC], i32)
        sb_idx1 = consts.tile([P, NCH], i32)
        sb_idx2 = consts.tile([P, NCH * S2], i32)
        sb_w5 = consts.tile([P, 10 * H], f32)
        sb_wfc = consts.tile([P, 2 * NCLS], f32)
        sb_bfc = consts.tile([P, NCLS], f32)
        sb_avgB = consts.tile([P, CPS * P], f32)
        sb_id = consts.tile([P, P], f32)
        nc.sync.dma_start(sb_idx0[:], idx0[:])
        nc.sync.dma_start(sb_idx1[:], idx1[:])
        nc.sync.dma_start(sb_idx2[:], idx2[:])
        nc.sync.dma_start(sb_w5[:], w5[:])
        nc.sync.dma_start(sb_wfc[:], wfc[:])
        nc.sync.dma_start(sb_bfc[:], bfc[:])
        nc.sync.dma_start(sb_avgB[:], avgB[:])
        nc.sync.dma_start(sb_id[:], ident[:])

        def wslot(k):
            return sb_w5[:, k * H:(k + 1) * H]

        # slots: wx1=0,1  wn1=2,3  wn1s=4,5  wx2=6,7  wn2=8,9

        def transpose256(src, tag_ps, tag_sb):
            """[128, 256] row-major -> [128, 256] where [:, 128k:...] holds
            the transpose of src's k-th feature half (i.e. feature-major)."""
            ps = tag_ps.tile([P, 2 * P], f32)
            nc.tensor.transpose(ps[:, 0:P], src[:, 0:P], sb_id[:])
            nc.tensor.transpose(ps[:, P:2 * P], src[:, P:2 * P], sb_id[:])
            sb = tag_sb.tile([P, 2 * P], f32)
            nc.scalar.copy(sb[:], ps[:])
            return sb

        def mm_pair(out_ps, xT, w0, w1, ctx_start=True):
            nc.tensor.matmul(out=out_ps, lhsT=xT[:, 0:P], rhs=w0,
                             start=True, stop=False)
            nc.tensor.matmul(out=out_ps, lhsT=xT[:, P:2 * P], rhs=w1,
                             start=False, stop=True)

        acc = None
        for s in range(NSC):
            acc = ps_acc.tile([P, 4 * P], f32, tag="acc")  # [agg1 | agg_a1]
            for j in range(CPS):
                c = s * CPS + j
                # ---- gathers ----
                h2t = p_h2.tile([P, S2 * D], f32, tag="h2")
                for q in range(S2):
                    # HW indirect DMA: one descriptor per partition (one
                    # offset, contiguous dest) -> one gather per neighbor.
                    gather(h2t[:, q * D:(q + 1) * D],
                           sb_idx2[:, c * S2 + q:c * S2 + q + 1], feats[:])
                ha = p_ha.tile([P, 2 * D], f32, tag="ha")  # [h1 | a1]
                gather(ha[:, 0:D], sb_idx1[:, c:c + 1], feats[:])
                # ---- agg2 = sum of the 10 neighbor rows ----
                agg2 = p_agg2.tile([P, D], f32, tag="agg2")
                nc.vector.tensor_add(agg2[:], h2t[:, 0:D], h2t[:, D:2 * D])
                for q in range(2, S2):
                    nc.vector.tensor_add(agg2[:], agg2[:],
                                         h2t[:, q * D:(q + 1) * D])
                # ---- transposes ----
                h1T = transpose256(ha[:, 0:D], ps_tr1, p_t)
                agg2T = transpose256(agg2[:], ps_tr2, p_t)
                # ---- a1 = relu([h1@wx1, agg2@wn1s]) ----
                a1ps = ps_mm.tile([P, D], f32, tag="a1ps")
                mm_pair(a1ps[:, 0:H], h1T, wslot(0), wslot(1))
                mm_pair(a1ps[:, H:D], agg2T, wslot(4), wslot(5))
                nc.scalar.activation(ha[:, D:2 * D], a1ps[:], AF.Relu)
                # ---- segment-mean accumulators (over 25 chunks) ----
                nc.tensor.matmul(
                    out=acc[:], lhsT=sb_avgB[:, j * P:(j + 1) * P], rhs=ha[:],
                    start=(j == 0), stop=(j == CPS - 1), skip_group_check=True,
                )

            # ---- supercycle tail: 128 roots ----
            h0t = p_misc.tile([P, D], f32, tag="h0")
            gather(h0t[:], sb_idx0[:, s:s + 1], feats[:])
            aggs = p_misc.tile([P, 2 * D], f32, tag="aggs")  # [agg1 | agg_a1]
            nc.vector.tensor_copy(aggs[:], acc[:])

            h0T = transpose256(h0t[:], ps_tr1, p_t)
            agg1T = transpose256(aggs[:, 0:D], ps_tr2, p_t)
            a0ps = ps_mm.tile([P, D], f32, tag="a1ps")
            mm_pair(a0ps[:, 0:H], h0T, wslot(0), wslot(1))
            mm_pair(a0ps[:, H:D], agg1T, wslot(2), wslot(3))
            a0t = p_misc.tile([P, D], f32, tag="a0")
            nc.scalar.activation(a0t[:], a0ps[:], AF.Relu)

            a0T = transpose256(a0t[:], ps_tr1, p_t)
            aggA1T = transpose256(aggs[:, D:2 * D], ps_tr2, p_t)
            b0ps = ps_mm.tile([P, D], f32, tag="a1ps")
            mm_pair(b0ps[:, 0:H], a0T, wslot(6), wslot(7))
            mm_pair(b0ps[:, H:D], aggA1T, wslot(8), wslot(9))
            b0t = p_misc.tile([P, D], f32, tag="b0")
            nc.scalar.activation(b0t[:], b0ps[:], AF.Relu)

            # ---- row-normalize (F.normalize with eps=1e-12) ----
            sq = p_misc.tile([P, D], f32, tag="sq")
            ss = p_misc.tile([P, 4], f32, tag="ss")
            nc.scalar.activation(sq[:], b0t[:], AF.Square,
                                 accum_out=ss[:, 0:1])
            nc.vector.tensor_scalar_max(ss[:, 1:2], ss[:, 0:1], 1e-24)
            nc.scalar.sqrt(ss[:, 2:3], ss[:, 1:2])
            nc.vector.reciprocal(ss[:, 3:4], ss[:, 2:3])
            b0n = p_misc.tile([P, D], f32, tag="b0n")
            nc.vector.tensor_scalar_mul(b0n[:], b0t[:], ss[:, 3:4])

            # ---- classifier ----
            b0nT = transpose256(b0n[:], ps_tr1, p_t)
            ops = ps_out.tile([P, NCLS], f32, tag="ops")
            nc.tensor.matmul(out=ops[:], lhsT=b0nT[:, 0:P],
                             rhs=sb_wfc[:, 0:NCLS], start=True, stop=False)
            nc.tensor.matmul(out=ops[:], lhsT=b0nT[:, P:2 * P],
                             rhs=sb_wfc[:, NCLS:2 * NCLS],
                             start=False, stop=True)
            osb = p_misc.tile([P, NCLS], f32, tag="osb")
            nc.vector.tensor_add(osb[:], ops[:], sb_bfc[:])
            nc.sync.dma_start(out[s * P:(s + 1) * P, :], osb[:])

    nc.compile()
    return nc


def _get_program(BC):
    if BC not in _programs:
        _programs[BC] = _build_program(BC)
    return _programs[BC]


def _host_prep_shared(wx1, wn1, wx2, wn2, w_fc, b_fc, CPS):
    """Weight/constant tensors shared by all cores, in SBUF-ready layouts."""
    def kmaj(w):  # [256,128] -> [128, 2, 128] halves of the K dim
        return np.ascontiguousarray(
            w.reshape(2, P, -1).transpose(1, 0, 2), np.float32)

    wn1s = wn1 * (1.0 / S2)   # a1 path: agg2 is a plain sum of 10
    w5 = np.concatenate(
        [kmaj(wx1), kmaj(wn1), kmaj(wn1s), kmaj(wx2), kmaj(wn2)], axis=1
    ).reshape(P, 10 * H)
    wfc = kmaj(w_fc).reshape(P, 2 * NCLS)
    bfc = np.ascontiguousarray(np.tile(b_fc.reshape(1, NCLS), (P, 1)), np.float32)
    # B_j averaging matrices: row p of chunk j belongs to local root
    # (128j+p)//25; entry 1/25 makes the accumulated matmul a mean over 25.
    avgB = np.zeros((P, CPS, P), np.float32)
    for j in range(CPS):
        for p in range(P):
            avgB[p, j, (P * j + p) // S1] = 1.0 / S1
    avgB = np.ascontiguousarray(avgB.reshape(P, CPS * P))
    ident = np.eye(P, dtype=np.float32)
    return w5, wfc, bfc, avgB, ident


def kernel(ids, ids_l1, ids_l2, feats, wx1, wn1, wx2, wn2, w_fc, b_fc):
    from concourse.bass_utils import run_bass_kernel_spmd

    ids = np.ascontiguousarray(np.asarray(ids), np.int32).ravel()
    ids_l1 = np.ascontiguousarray(np.asarray(ids_l1), np.int32).ravel()
    ids_l2 = np.ascontiguousarray(np.asarray(ids_l2), np.int32).ravel()
    feats = np.ascontiguousarray(np.asarray(feats), np.float32)
    wx1, wn1, wx2, wn2, w_fc, b_fc = (
        np.asarray(a, np.float32) for a in (wx1, wn1, wx2, wn2, w_fc, b_fc))

    B = ids.shape[0]
    BC = B // NCORES
    L1, L2 = BC * S1, BC * S1 * S2
    NCH = L1 // P
    NSC = BC // P
    CPS = NCH // NSC

    w5, wfc, bfc, avgB, ident = _host_prep_shared(
        wx1, wn1, wx2, wn2, w_fc, b_fc, CPS)

    in_maps = []
    for c in range(NCORES):
        i0 = ids[c * BC:(c + 1) * BC]
        i1 = ids_l1[c * L1:(c + 1) * L1]
        i2 = ids_l2[c * L2:(c + 1) * L2]
        in_maps.append({
            "feats": feats,
            "idx0": np.ascontiguousarray(i0.reshape(NSC, P).T),
            "idx1": np.ascontiguousarray(i1.reshape(NCH, P).T),
            "idx2": np.ascontiguousarray(
                i2.reshape(NCH, P, S2).transpose(1, 0, 2).reshape(P, NCH * S2)),
            "w5": w5, "wfc": wfc, "bfc": bfc, "avgB": avgB, "ident": ident,
        })

    nc = _get_program(BC)
    res = run_bass_kernel_spmd(
        nc, in_maps, list(range(NCORES)),
        trace=bool(os.environ.get("KERNEL_TRACE")),
    )
    global LAST_RESULTS
    LAST_RESULTS = res
    return np.concatenate(
        [res.results[c]["out"] for c in range(NCORES)], axis=0
    ).astype(np.float32)


LAST_RESULTS = None

